# revision 11
# baseline (speedup 1.0000x reference)
"""Distributed causal multi-head attention block for Trainium2 (8 NeuronCores).

Problem: x[2,2048,1024] -> c_attn(QKV) -> 16-head causal attention -> c_proj.

Sharding (hardcoded): DP=2 on batch x TP=4 on heads. Core c handles
batch b=c//4 and heads 4*(c%4)..4*(c%4)+3. Each core computes Q^T,K^T
(hidden-transposed layout), V (natural layout, with an appended ones
column for softmax row sums), flash-style causal attention with scores
kept transposed [keys, queries] so no on-device transposes are needed,
then normalizes z by the softmax row sums. z^T shards ([256,2048] bf16)
are AllGather'd within each batch group of 4 cores, and each core
computes the c_proj for a 256-wide slice of the output-feature axis
(the w_proj column slice is baked into that core's input), so the final
host-side step is a pure concatenation.

Compute dtype bf16 on the TensorEngine, f32 softmax statistics.
x is pre-transposed/cast on the host (input marshalling) so the device
graph has zero transposes.
"""
import contextlib
import ctypes
import os
import sys
import types

import numpy as np

# ---------------------------------------------------------------- problem dims
B, S, D = 2, 2048, 1024
H, HD = 16, 64
N_CORES = 8
TP = 4                   # cores per batch group (head-parallel)
HPC = H // TP            # heads per core = 4
QCOLS = HPC * HD         # 256 q (and k, v) columns per core
ESL = D // TP            # 256 output-feature columns per core
KC = D // 128            # 8 contraction chunks
NKT = S // 128           # 16 key tiles
NQT = S // 512           # 4 query tiles (512-wide, free dim)
GROUPS = [[0, 1, 2, 3], [4, 5, 6, 7]]


def _install_ntff_shim():
    """Make `antenv.axon_hooks` importable so BASS_TRACE profiling works."""
    if "antenv.axon_hooks" in sys.modules:
        return
    try:
        lib = ctypes.CDLL("/opt/axon/libaxon_pjrt.so")
        lib.axon_start_nrt_profile.argtypes = [ctypes.POINTER(ctypes.c_int64), ctypes.c_size_t]
        lib.axon_start_nrt_profile.restype = ctypes.c_int64
        lib.axon_stop_nrt_profile.argtypes = [ctypes.c_char_p]
        lib.axon_stop_nrt_profile.restype = ctypes.c_int64
    except (OSError, AttributeError):
        lib = None

    @contextlib.contextmanager
    def _hook(output_dir, device_ids):
        import jax
        jax.devices()
        if device_ids:
            ids = (ctypes.c_int64 * len(device_ids))(*device_ids)
            rc = lib.axon_start_nrt_profile(ids, len(device_ids))
        else:
            rc = lib.axon_start_nrt_profile(None, 0)
        if rc != 0:
            raise RuntimeError(f"axon_start_nrt_profile rc={rc}")
        try:
            yield
        finally:
            n = lib.axon_stop_nrt_profile(str(output_dir).encode())
            print(f"profile: {n} file(s) written to {output_dir}", file=sys.stderr)

    mod = types.ModuleType("antenv.axon_hooks")
    mod.get_axon_ntff_profile_hook = lambda: (_hook if lib is not None else None)
    mod.set_axon_ntff_profile_hook = lambda h: None
    sys.modules["antenv.axon_hooks"] = mod


_install_ntff_shim()

import concourse.bacc as bacc
import concourse.mybir as mybir
import concourse.tile as tile
from concourse.bass_utils import run_bass_kernel_spmd

F32 = mybir.dt.float32
BF16 = mybir.dt.bfloat16
NPBF16 = np.dtype(mybir.dt.np(BF16))
EXP = mybir.ActivationFunctionType.Exp
MUL = mybir.AluOpType.mult
ADD = mybir.AluOpType.add


def build_graph():
    nc = bacc.Bacc("TRN2", target_bir_lowering=False, debug=False,
                   enable_asserts=True, num_devices=N_CORES)

    xT_d = nc.dram_tensor("xT", [D, S], BF16, kind="ExternalInput")
    wqk_d = nc.dram_tensor("wqk", [D, 2 * QCOLS], BF16, kind="ExternalInput")
    wv_d = nc.dram_tensor("wv", [D, QCOLS], BF16, kind="ExternalInput")
    wp_d = nc.dram_tensor("wp", [D, ESL], BF16, kind="ExternalInput")
    bqk_d = nc.dram_tensor("bqk", [128, 4], F32, kind="ExternalInput")
    bv_d = nc.dram_tensor("bv", [128, QCOLS], F32, kind="ExternalInput")
    bp_d = nc.dram_tensor("bp", [128, ESL], F32, kind="ExternalInput")
    tri_d = nc.dram_tensor("tri", [128, 128], BF16, kind="ExternalInput")
    ones_d = nc.dram_tensor("ones64", [65, HD], F32, kind="ExternalInput")
    out_d = nc.dram_tensor("out", [S, ESL], F32, kind="ExternalOutput")

    with tile.TileContext(nc) as tc:
        with (
            tc.tile_pool(name="sb", bufs=1) as sb,
            tc.tile_pool(name="pt", bufs=3) as ptp,
            tc.tile_pool(name="ob", bufs=3) as obp,
            tc.tile_pool(name="psA", bufs=3, space="PSUM") as psA,
            tc.tile_pool(name="psZ", bufs=2, space="PSUM") as psZ,
            tc.tile_pool(name="dram", bufs=1, space="DRAM") as dram,
        ):
            # ---------------- persistent SBUF tensors ----------------
            xT_sb = sb.tile([128, KC * S], BF16, tag="xT")
            wqk_sb = sb.tile([128, KC * 2 * QCOLS], BF16, tag="wqk")
            wv_sb = sb.tile([128, KC * QCOLS], BF16, tag="wv")
            wp_sb = sb.tile([128, KC * ESL], BF16, tag="wp")
            qT_sb = sb.tile([128, 2 * S], BF16, tag="qT")
            kT_sb = sb.tile([128, 2 * S], BF16, tag="kT")
            v_sb = sb.tile([128, NKT * HPC * (HD + 1)], BF16, tag="v")
            zaug_sb = sb.tile([HD, HPC * NQT * 512], F32, tag="zaug")
            # softmax row sums, packed on matmul-legal partitions {0,32,64}:
            # idx -> (row 32*(idx%3), cols (idx//3)*512). r_sum ends up
            # holding 1/r after the Ln/Exp pair below.
            r_sum = sb.tile([65, 6 * 512], F32, tag="rsum")
            r_tmp = sb.tile([65, 6 * 512], F32, tag="rtmp")
            z_sb = sb.tile([128, 2 * S], BF16, tag="z")
            zg_sb = sb.tile([128, KC * S], BF16, tag="zg")
            bqk_sb = sb.tile([128, 4], F32, tag="bqk")
            bv_sb = sb.tile([128, QCOLS], F32, tag="bv")
            bp_sb = sb.tile([128, ESL], F32, tag="bp")
            tri_sb = sb.tile([128, 128], BF16, tag="tri")
            ones_sb = sb.tile([65, HD], F32, tag="ones")

            # ---------------- input DMAs ----------------
            for k in range(KC):
                nc.sync.dma_start(out=xT_sb[:, k * S:(k + 1) * S],
                                  in_=xT_d[k * 128:(k + 1) * 128, :])
            for k in range(KC):
                nc.sync.dma_start(out=wqk_sb[:, k * 512:(k + 1) * 512],
                                  in_=wqk_d[k * 128:(k + 1) * 128, :])
                nc.sync.dma_start(out=wv_sb[:, k * QCOLS:(k + 1) * QCOLS],
                                  in_=wv_d[k * 128:(k + 1) * 128, :])
                nc.sync.dma_start(out=wp_sb[:, k * ESL:(k + 1) * ESL],
                                  in_=wp_d[k * 128:(k + 1) * 128, :])
            nc.sync.dma_start(out=bqk_sb[:], in_=bqk_d[:])
            nc.sync.dma_start(out=bv_sb[:], in_=bv_d[:])
            nc.sync.dma_start(out=bp_sb[:], in_=bp_d[:])
            nc.sync.dma_start(out=tri_sb[:], in_=tri_d[:])
            nc.sync.dma_start(out=ones_sb[:], in_=ones_d[:])

            # ones columns of V_aug (overwritten below except col 64 per head)
            nc.vector.memset(v_sb[:], 1.0)

            # ---------------- phase A: QKV projections ----------------
            # Q^T/K^T: hidden^T orientation  psum[cols128, tok512]
            for nt in range(NQT):
                for mc in range(4):          # 0,1 -> Q head pairs; 2,3 -> K
                    ps = psA.tile([128, 512], F32, tag="m")
                    for k in range(KC):
                        nc.tensor.matmul(
                            ps[:],
                            lhsT=wqk_sb[:, k * 512 + mc * 128: k * 512 + (mc + 1) * 128],
                            rhs=xT_sb[:, k * S + nt * 512: k * S + (nt + 1) * 512],
                            start=(k == 0), stop=(k == KC - 1))
                    dst = qT_sb if mc < 2 else kT_sb
                    c2 = mc % 2
                    nc.vector.tensor_scalar_add(
                        dst[:, c2 * S + nt * 512: c2 * S + (nt + 1) * 512],
                        ps[:], bqk_sb[:, mc:mc + 1])
                # V natural orientation for the 4 token-128 tiles of this nt
                for tt in range(4):
                    t = nt * 4 + tt
                    psv = psA.tile([128, QCOLS], F32, tag="m")
                    for k in range(KC):
                        nc.tensor.matmul(
                            psv[:],
                            lhsT=xT_sb[:, k * S + t * 128: k * S + (t + 1) * 128],
                            rhs=wv_sb[:, k * QCOLS:(k + 1) * QCOLS],
                            start=(k == 0), stop=(k == KC - 1))
                    vdst = v_sb[:].rearrange(
                        "p (t h e) -> p t h e", t=NKT, e=HD + 1)[:, t, :, 0:HD]
                    nc.vector.tensor_tensor(
                        vdst,
                        psv[:].rearrange("p (h d) -> p h d", h=HPC),
                        bv_sb[:].rearrange("p (h d) -> p h d", h=HPC),
                        ADD)

            # ---------------- phase B: causal attention ----------------
            for h in range(HPC):
                hp, ho = h // 2, (h % 2) * HD          # head-pair chunk, partition offset
                for qt in range(NQT):
                    q0 = qt * 512
                    n_kt = 4 * qt + 4
                    idx = h * NQT + qt
                    zaug = psZ.tile([HD + 1, 512], F32, tag="z")
                    for g in range(n_kt // 2):
                        kts = (2 * g, 2 * g + 1)
                        st = psA.tile([128, 1024], F32, tag="m")
                        pT = ptp.tile([128, 1024], BF16, tag="pT")
                        widths = []
                        for j, kt in enumerate(kts):
                            k0 = kt * 128
                            qstart = max(q0, k0)
                            w = q0 + 512 - qstart
                            widths.append((kt, k0, qstart, w))
                            nc.tensor.matmul(
                                st[:, j * 512: j * 512 + w],
                                lhsT=kT_sb[ho:ho + HD, hp * S + k0: hp * S + k0 + 128],
                                rhs=qT_sb[ho:ho + HD, hp * S + qstart: hp * S + qstart + w],
                                start=True, stop=True)
                        ext = 512 + widths[1][3]
                        nc.scalar.activation(pT[:, 0:ext], st[:, 0:ext], EXP, scale=0.125)
                        for j, (kt, k0, qstart, w) in enumerate(widths):
                            if k0 >= q0:   # diagonal tile: causal triangle mask
                                nc.vector.tensor_tensor(
                                    pT[:, j * 512: j * 512 + 128],
                                    pT[:, j * 512: j * 512 + 128],
                                    tri_sb[:], MUL)
                        for j, (kt, k0, qstart, w) in enumerate(widths):
                            nc.tensor.matmul(
                                zaug[:, qstart - q0: 512],
                                lhsT=v_sb[:, kt * HPC * (HD + 1) + h * (HD + 1):
                                          kt * HPC * (HD + 1) + (h + 1) * (HD + 1)],
                                rhs=pT[:, j * 512: j * 512 + w],
                                start=(kt == 0), stop=(kt == n_kt - 1))
                    nc.vector.tensor_copy(zaug_sb[:, idx * 512:(idx + 1) * 512],
                                          zaug[0:HD, :])
                    ro, co = 32 * (idx % 3), (idx // 3) * 512
                    nc.vector.tensor_copy(r_sum[ro:ro + 1, co:co + 512],
                                          zaug[HD:HD + 1, :])

            # ---------------- phase B': softmax normalization ----------------
            # 1/r via exp(-ln(r)); Ln and Exp share one ACT table set, and the
            # DVE reciprocal would cost 8 cycles per free element on one lane.
            nc.scalar.activation(r_tmp[:], r_sum[:], mybir.ActivationFunctionType.Ln)
            nc.scalar.activation(r_sum[:], r_tmp[:], EXP, scale=-1.0)
            for h in range(HPC):
                zc, zo = h // 2, (h % 2) * HD
                for qt in range(NQT):
                    idx = h * NQT + qt
                    ro, co = 32 * (idx % 3), (idx // 3) * 512
                    rbc = psZ.tile([HD, 512], F32, tag="z")
                    nc.tensor.matmul(rbc[:], lhsT=ones_sb[ro:ro + 1, :],
                                     rhs=r_sum[ro:ro + 1, co:co + 512],
                                     start=True, stop=True)
                    nc.vector.tensor_tensor(
                        z_sb[zo:zo + HD, zc * S + qt * 512: zc * S + (qt + 1) * 512],
                        zaug_sb[:, idx * 512:(idx + 1) * 512], rbc[:], MUL)

            # ---------------- phase C: AllGather z across the batch group ----
            z_dram = dram.tile([2 * 128, S], BF16, tag="zd")
            zg_dram = dram.tile([TP * 2 * 128, S], BF16, tag="zgd")
            for c2 in range(2):
                nc.sync.dma_start(out=z_dram[c2 * 128:(c2 + 1) * 128, :],
                                  in_=z_sb[:, c2 * S:(c2 + 1) * S])
            nc.gpsimd.collective_compute(
                "AllGather", mybir.AluOpType.bypass, replica_groups=GROUPS,
                ins=[z_dram.opt()], outs=[zg_dram.opt()])
            for k in range(KC):
                nc.sync.dma_start(out=zg_sb[:, k * S:(k + 1) * S],
                                  in_=zg_dram[k * 128:(k + 1) * 128, :])

            # ---------------- phase D: c_proj (output-feature slice) --------
            for mt in range(NKT):
                po = psA.tile([128, ESL], F32, tag="m")
                for k in range(KC):
                    nc.tensor.matmul(
                        po[:],
                        lhsT=zg_sb[:, k * S + mt * 128: k * S + (mt + 1) * 128],
                        rhs=wp_sb[:, k * ESL:(k + 1) * ESL],
                        start=(k == 0), stop=(k == KC - 1))
                o_sb = obp.tile([128, ESL], F32, tag="o")
                nc.vector.tensor_tensor(o_sb[:], po[:], bp_sb[:], ADD)
                nc.sync.dma_start(out=out_d[mt * 128:(mt + 1) * 128, :], in_=o_sb[:])

    nc.compile()
    return nc


_NC = None


def _get_nc():
    global _NC
    if _NC is None:
        _NC = build_graph()
    return _NC


def _make_in_maps(x, w_attn, b_attn, w_proj, b_proj):
    x = np.asarray(x, dtype=np.float32)
    w_attn = np.asarray(w_attn, dtype=np.float32)
    b_attn = np.asarray(b_attn, dtype=np.float32)
    w_proj = np.asarray(w_proj, dtype=np.float32)
    b_proj = np.asarray(b_proj, dtype=np.float32)

    tri = np.triu(np.ones((128, 128), np.float32)).astype(NPBF16)  # tri[k,j]=1 iff j>=k
    ones64 = np.ones((65, HD), np.float32)
    xT = [np.ascontiguousarray(x[b].T).astype(NPBF16) for b in range(B)]

    in_maps = []
    for c in range(N_CORES):
        b, hg = c // TP, c % TP
        qs, ks, vs = hg * QCOLS, D + hg * QCOLS, 2 * D + hg * QCOLS
        es = (c % TP) * ESL
        wqk = np.concatenate(
            [w_attn[:, qs:qs + QCOLS], w_attn[:, ks:ks + QCOLS]], axis=1
        ).astype(NPBF16)
        wv = np.ascontiguousarray(w_attn[:, vs:vs + QCOLS]).astype(NPBF16)
        wp = np.ascontiguousarray(w_proj[:, es:es + ESL]).astype(NPBF16)
        bqk = np.stack([b_attn[qs:qs + 128], b_attn[qs + 128:qs + QCOLS],
                        b_attn[ks:ks + 128], b_attn[ks + 128:ks + QCOLS]],
                       axis=1).astype(np.float32)
        bv = np.ascontiguousarray(
            np.broadcast_to(b_attn[vs:vs + QCOLS], (128, QCOLS))).astype(np.float32)
        bp = np.ascontiguousarray(
            np.broadcast_to(b_proj[es:es + ESL], (128, ESL))).astype(np.float32)
        in_maps.append({
            "xT": xT[b], "wqk": wqk, "wv": wv, "wp": wp,
            "bqk": bqk, "bv": bv, "bp": bp, "tri": tri, "ones64": ones64,
        })
    return in_maps


def kernel(x, w_attn, b_attn, w_proj, b_proj):
    nc = _get_nc()
    in_maps = _make_in_maps(x, w_attn, b_attn, w_proj, b_proj)
    res = run_bass_kernel_spmd(nc, in_maps, core_ids=list(range(N_CORES)),
                               trace=bool(os.environ.get("BASS_TRACE")))
    if res.exec_time_ns is not None:
        print(f"HW exec time: {res.exec_time_ns} ns")
    out = np.empty((B, S, D), np.float32)
    for c in range(N_CORES):
        b, es = c // TP, (c % TP) * ESL
        out[b, :, es:es + ESL] = res.results[c]["out"]
    return out


# revision 15
# speedup vs baseline: 1.1164x; 1.1164x over previous
"""Distributed causal multi-head attention block for Trainium2 (8 NeuronCores).

Problem: x[2,2048,1024] -> c_attn(QKV) -> 16-head causal attention -> c_proj.

Sharding (hardcoded): DP=2 on batch x TP=4 on heads. Core c handles
batch b=c//4 and heads 4*(c%4)..4*(c%4)+3. Each core computes Q^T,K^T
(hidden-transposed layout), V (natural layout, with an appended ones
column for softmax row sums), flash-style causal attention with scores
kept transposed [keys, queries] so no on-device transposes are needed,
then normalizes z by the softmax row sums. z^T shards ([256,2048] bf16)
are AllGather'd within each batch group of 4 cores, and each core
computes the c_proj for a 256-wide slice of the output-feature axis
(the w_proj column slice is baked into that core's input), so the final
host-side step is a pure concatenation.

Compute dtype bf16 on the TensorEngine, f32 softmax statistics.
x is pre-transposed/cast on the host (input marshalling) so the device
graph has zero transposes.
"""
import contextlib
import ctypes
import os
import sys
import types

import numpy as np

# ---------------------------------------------------------------- problem dims
B, S, D = 2, 2048, 1024
H, HD = 16, 64
N_CORES = 8
TP = 4                   # cores per batch group (head-parallel)
HPC = H // TP            # heads per core = 4
QCOLS = HPC * HD         # 256 q (and k, v) columns per core
ESL = D // TP            # 256 output-feature columns per core
KC = D // 128            # 8 contraction chunks
NKT = S // 128           # 16 key tiles
NQT = S // 512           # 4 query tiles (512-wide, free dim)
GROUPS = [[0, 1, 2, 3], [4, 5, 6, 7]]


def _install_ntff_shim():
    """Make `antenv.axon_hooks` importable so BASS_TRACE profiling works."""
    if "antenv.axon_hooks" in sys.modules:
        return
    try:
        lib = ctypes.CDLL("/opt/axon/libaxon_pjrt.so")
        lib.axon_start_nrt_profile.argtypes = [ctypes.POINTER(ctypes.c_int64), ctypes.c_size_t]
        lib.axon_start_nrt_profile.restype = ctypes.c_int64
        lib.axon_stop_nrt_profile.argtypes = [ctypes.c_char_p]
        lib.axon_stop_nrt_profile.restype = ctypes.c_int64
    except (OSError, AttributeError):
        lib = None

    @contextlib.contextmanager
    def _hook(output_dir, device_ids):
        import jax
        jax.devices()
        if device_ids:
            ids = (ctypes.c_int64 * len(device_ids))(*device_ids)
            rc = lib.axon_start_nrt_profile(ids, len(device_ids))
        else:
            rc = lib.axon_start_nrt_profile(None, 0)
        if rc != 0:
            raise RuntimeError(f"axon_start_nrt_profile rc={rc}")
        try:
            yield
        finally:
            n = lib.axon_stop_nrt_profile(str(output_dir).encode())
            print(f"profile: {n} file(s) written to {output_dir}", file=sys.stderr)

    mod = types.ModuleType("antenv.axon_hooks")
    mod.get_axon_ntff_profile_hook = lambda: (_hook if lib is not None else None)
    mod.set_axon_ntff_profile_hook = lambda h: None
    sys.modules["antenv.axon_hooks"] = mod


_install_ntff_shim()

import concourse.bacc as bacc
import concourse.mybir as mybir
import concourse.tile as tile
from concourse.bass_utils import run_bass_kernel_spmd

F32 = mybir.dt.float32
BF16 = mybir.dt.bfloat16
NPBF16 = np.dtype(mybir.dt.np(BF16))
EXP = mybir.ActivationFunctionType.Exp
MUL = mybir.AluOpType.mult
ADD = mybir.AluOpType.add


def build_graph():
    nc = bacc.Bacc("TRN2", target_bir_lowering=False, debug=False,
                   enable_asserts=True, num_devices=N_CORES)

    xT_d = nc.dram_tensor("xT", [D, S], BF16, kind="ExternalInput")
    wqk_d = nc.dram_tensor("wqk", [D, 2 * QCOLS], BF16, kind="ExternalInput")
    wv_d = nc.dram_tensor("wv", [D, QCOLS], BF16, kind="ExternalInput")
    wp_d = nc.dram_tensor("wp", [D, ESL], BF16, kind="ExternalInput")
    bqk_d = nc.dram_tensor("bqk", [128, 4], F32, kind="ExternalInput")
    bv_d = nc.dram_tensor("bv", [128, QCOLS], F32, kind="ExternalInput")
    bp_d = nc.dram_tensor("bp", [128, ESL], F32, kind="ExternalInput")
    tri_d = nc.dram_tensor("tri", [128, 128], BF16, kind="ExternalInput")
    ones_d = nc.dram_tensor("ones64", [65, HD], BF16, kind="ExternalInput")
    out_d = nc.dram_tensor("out", [S, ESL], F32, kind="ExternalOutput")

    with tile.TileContext(nc) as tc:
        with (
            tc.tile_pool(name="sb", bufs=1) as sb,
            tc.tile_pool(name="pt", bufs=3) as ptp,
            tc.tile_pool(name="ob", bufs=3) as obp,
            tc.tile_pool(name="psA", bufs=3, space="PSUM") as psA,
            tc.tile_pool(name="psZ", bufs=2, space="PSUM") as psZ,
            tc.tile_pool(name="dram", bufs=2, space="DRAM") as dram,
        ):
            # ---------------- persistent SBUF tensors ----------------
            xT_sb = sb.tile([128, KC * S], BF16, tag="xT")
            wqk_sb = sb.tile([128, KC * 2 * QCOLS], BF16, tag="wqk")
            wv_sb = sb.tile([128, KC * QCOLS], BF16, tag="wv")
            wp_sb = sb.tile([128, KC * ESL], BF16, tag="wp")
            qT_sb = sb.tile([128, 2 * S], BF16, tag="qT")
            kT_sb = sb.tile([128, 2 * S], BF16, tag="kT")
            v_sb = sb.tile([128, NKT * HPC * (HD + 1)], BF16, tag="v")
            zaug_sb = sb.tile([HD, HPC * NQT * 512], F32, tag="zaug")
            # softmax row sums, packed on matmul-legal partitions {0,32}:
            # (h,qt) -> (row 32*(qt%2), cols h*1024 + 512*(qt//2)), so each
            # head's four sums live in one [65, 1024] block for per-head Ln/Exp.
            r_sum = sb.tile([65, HPC * 1024], F32, tag="rsum")
            r_tmp = sb.tile([65, HPC * 1024], F32, tag="rtmp")
            r_invb = sb.tile([65, HPC * 1024], BF16, tag="rinvb")
            z_sb = sb.tile([128, 2 * S], BF16, tag="z")
            zg_sb = sb.tile([128, KC * S], BF16, tag="zg")
            bqk_sb = sb.tile([128, 4], F32, tag="bqk")
            bv_sb = sb.tile([128, QCOLS], F32, tag="bv")
            bp_sb = sb.tile([128, ESL], F32, tag="bp")
            tri_sb = sb.tile([128, 128], BF16, tag="tri")
            ones_sb = sb.tile([65, HD], BF16, tag="ones")

            # ---------------- input DMAs (weights first, then x) ----------------
            for k in range(KC):
                nc.sync.dma_start(out=wqk_sb[:, k * 512:(k + 1) * 512],
                                  in_=wqk_d[k * 128:(k + 1) * 128, :])
                nc.sync.dma_start(out=wv_sb[:, k * QCOLS:(k + 1) * QCOLS],
                                  in_=wv_d[k * 128:(k + 1) * 128, :])
            for k in range(KC):
                nc.sync.dma_start(out=xT_sb[:, k * S:(k + 1) * S],
                                  in_=xT_d[k * 128:(k + 1) * 128, :])
            for k in range(KC):
                nc.sync.dma_start(out=wp_sb[:, k * ESL:(k + 1) * ESL],
                                  in_=wp_d[k * 128:(k + 1) * 128, :])
            nc.sync.dma_start(out=bqk_sb[:], in_=bqk_d[:])
            nc.sync.dma_start(out=bv_sb[:], in_=bv_d[:])
            nc.sync.dma_start(out=bp_sb[:], in_=bp_d[:])
            nc.sync.dma_start(out=tri_sb[:], in_=tri_d[:])
            nc.sync.dma_start(out=ones_sb[:], in_=ones_d[:])

            # ones columns of V_aug (overwritten below except col 64 per head)
            nc.vector.memset(v_sb[:], 1.0)

            # ---------------- phase A: QKV projections ----------------
            # Q^T/K^T: hidden^T orientation  psum[cols128, tok512]
            for nt in range(NQT):
                for mc in range(4):          # 0,1 -> Q head pairs; 2,3 -> K
                    ps = psA.tile([128, 512], F32, tag="m")
                    for k in range(KC):
                        nc.tensor.matmul(
                            ps[:],
                            lhsT=wqk_sb[:, k * 512 + mc * 128: k * 512 + (mc + 1) * 128],
                            rhs=xT_sb[:, k * S + nt * 512: k * S + (nt + 1) * 512],
                            start=(k == 0), stop=(k == KC - 1))
                    dst = qT_sb if mc < 2 else kT_sb
                    c2 = mc % 2
                    nc.vector.tensor_scalar_add(
                        dst[:, c2 * S + nt * 512: c2 * S + (nt + 1) * 512],
                        ps[:], bqk_sb[:, mc:mc + 1])
                # V natural orientation for the 4 token-128 tiles of this nt
                for tt in range(4):
                    t = nt * 4 + tt
                    psv = psA.tile([128, QCOLS], F32, tag="m")
                    for k in range(KC):
                        nc.tensor.matmul(
                            psv[:],
                            lhsT=xT_sb[:, k * S + t * 128: k * S + (t + 1) * 128],
                            rhs=wv_sb[:, k * QCOLS:(k + 1) * QCOLS],
                            start=(k == 0), stop=(k == KC - 1))
                    vdst = v_sb[:].rearrange(
                        "p (t h e) -> p t h e", t=NKT, e=HD + 1)[:, t, :, 0:HD]
                    nc.vector.tensor_tensor(
                        vdst,
                        psv[:].rearrange("p (h d) -> p h d", h=HPC),
                        bv_sb[:].rearrange("p (h d) -> p h d", h=HPC),
                        ADD)

            # ---------------- phase B: causal attention ----------------
            for h in range(HPC):
                hp, ho = h // 2, (h % 2) * HD          # head-pair chunk, partition offset
                for qt in range(NQT):
                    q0 = qt * 512
                    n_kt = 4 * qt + 4
                    idx = h * NQT + qt
                    zaug = psZ.tile([HD + 1, 512], F32, tag="z")
                    for g in range(n_kt // 2):
                        kts = (2 * g, 2 * g + 1)
                        st = psA.tile([128, 1024], F32, tag="m")
                        pT = ptp.tile([128, 1024], BF16, tag="pT")
                        widths = []
                        for j, kt in enumerate(kts):
                            k0 = kt * 128
                            qstart = max(q0, k0)
                            w = q0 + 512 - qstart
                            widths.append((kt, k0, qstart, w))
                            nc.tensor.matmul(
                                st[:, j * 512: j * 512 + w],
                                lhsT=kT_sb[ho:ho + HD, hp * S + k0: hp * S + k0 + 128],
                                rhs=qT_sb[ho:ho + HD, hp * S + qstart: hp * S + qstart + w],
                                start=True, stop=True)
                        ext = 512 + widths[1][3]
                        nc.scalar.activation(pT[:, 0:ext], st[:, 0:ext], EXP, scale=0.125)
                        for j, (kt, k0, qstart, w) in enumerate(widths):
                            if k0 >= q0:   # diagonal tile: causal triangle mask
                                nc.vector.tensor_tensor(
                                    pT[:, j * 512: j * 512 + 128],
                                    pT[:, j * 512: j * 512 + 128],
                                    tri_sb[:], MUL)
                        for j, (kt, k0, qstart, w) in enumerate(widths):
                            nc.tensor.matmul(
                                zaug[:, qstart - q0: 512],
                                lhsT=v_sb[:, kt * HPC * (HD + 1) + h * (HD + 1):
                                          kt * HPC * (HD + 1) + (h + 1) * (HD + 1)],
                                rhs=pT[:, j * 512: j * 512 + w],
                                start=(kt == 0), stop=(kt == n_kt - 1))
                    nc.vector.tensor_copy(zaug_sb[:, idx * 512:(idx + 1) * 512],
                                          zaug[0:HD, :])
                    ro, co = 32 * (qt % 2), h * 1024 + 512 * (qt // 2)
                    nc.vector.tensor_copy(r_sum[ro:ro + 1, co:co + 512],
                                          zaug[HD:HD + 1, :])

                # ---- per-head softmax normalization + chunked AllGather ----
                # 1/r via exp(-ln(r)); Ln and Exp share one ACT table set and
                # the DVE reciprocal costs 8 cycles per free element.
                hc = slice(h * 1024, (h + 1) * 1024)
                nc.scalar.activation(r_tmp[:, hc], r_sum[:, hc],
                                     mybir.ActivationFunctionType.Ln)
                nc.scalar.activation(r_invb[:, hc], r_tmp[:, hc], EXP, scale=-1.0)
                for qt in range(NQT):
                    idx = h * NQT + qt
                    ro, co = 32 * (qt % 2), h * 1024 + 512 * (qt // 2)
                    rbc = psZ.tile([HD, 512], F32, tag="z")
                    nc.tensor.matmul(rbc[:], lhsT=ones_sb[ro:ro + 1, :],
                                     rhs=r_invb[ro:ro + 1, co:co + 512],
                                     start=True, stop=True)
                    nc.vector.tensor_tensor(
                        z_sb[ho:ho + HD, hp * S + qt * 512: hp * S + (qt + 1) * 512],
                        zaug_sb[:, idx * 512:(idx + 1) * 512], rbc[:], MUL)
                # AllGather this head's z across the batch group; the gathered
                # rows are head h of cores 0..3 = global heads {h,h+4,h+8,h+12}
                # (w_proj rows are permuted host-side to match).
                zd = dram.tile([HD, S], BF16, tag="zd")
                zgd = dram.tile([TP * HD, S], BF16, tag="zgd")
                nc.sync.dma_start(out=zd[:], in_=z_sb[ho:ho + HD, hp * S:(hp + 1) * S])
                nc.gpsimd.collective_compute(
                    "AllGather", mybir.AluOpType.bypass, replica_groups=GROUPS,
                    ins=[zd.opt()], outs=[zgd.opt()])
                for j in range(2):
                    nc.sync.dma_start(
                        out=zg_sb[:, (2 * h + j) * S:(2 * h + j + 1) * S],
                        in_=zgd[j * 128:(j + 1) * 128, :])

            # ---------------- phase D: c_proj (output-feature slice) --------
            for mt in range(NKT):
                po = psA.tile([128, ESL], F32, tag="m")
                for k in range(KC):
                    nc.tensor.matmul(
                        po[:],
                        lhsT=zg_sb[:, k * S + mt * 128: k * S + (mt + 1) * 128],
                        rhs=wp_sb[:, k * ESL:(k + 1) * ESL],
                        start=(k == 0), stop=(k == KC - 1))
                o_sb = obp.tile([128, ESL], F32, tag="o")
                nc.vector.tensor_tensor(o_sb[:], po[:], bp_sb[:], ADD)
                nc.sync.dma_start(out=out_d[mt * 128:(mt + 1) * 128, :], in_=o_sb[:])

    nc.compile()
    return nc


_NC = None


def _get_nc():
    global _NC
    if _NC is None:
        _NC = build_graph()
    return _NC


def _make_in_maps(x, w_attn, b_attn, w_proj, b_proj):
    x = np.asarray(x, dtype=np.float32)
    w_attn = np.asarray(w_attn, dtype=np.float32)
    b_attn = np.asarray(b_attn, dtype=np.float32)
    w_proj = np.asarray(w_proj, dtype=np.float32)
    b_proj = np.asarray(b_proj, dtype=np.float32)

    tri = np.triu(np.ones((128, 128), np.float32)).astype(NPBF16)  # tri[k,j]=1 iff j>=k
    ones64 = np.ones((65, HD), np.float32).astype(NPBF16)
    xT = [np.ascontiguousarray(x[b].T).astype(NPBF16) for b in range(B)]

    in_maps = []
    for c in range(N_CORES):
        b, hg = c // TP, c % TP
        qs, ks, vs = hg * QCOLS, D + hg * QCOLS, 2 * D + hg * QCOLS
        es = (c % TP) * ESL
        wqk = np.concatenate(
            [w_attn[:, qs:qs + QCOLS], w_attn[:, ks:ks + QCOLS]], axis=1
        ).astype(NPBF16)
        wv = np.ascontiguousarray(w_attn[:, vs:vs + QCOLS]).astype(NPBF16)
        # zg_sb chunk k=2h+j holds (heads h+4*(2j), h+4*(2j+1)) x 64 dims;
        # permute w_proj rows to match the gathered layout.
        perm = np.empty(D, np.int64)
        for k in range(KC):
            hh, j = k // 2, k % 2
            for p in range(128):
                perm[k * 128 + p] = (hh + 4 * (2 * j + p // HD)) * HD + p % HD
        wp = np.ascontiguousarray(w_proj[perm][:, es:es + ESL]).astype(NPBF16)
        bqk = np.stack([b_attn[qs:qs + 128], b_attn[qs + 128:qs + QCOLS],
                        b_attn[ks:ks + 128], b_attn[ks + 128:ks + QCOLS]],
                       axis=1).astype(np.float32)
        bv = np.ascontiguousarray(
            np.broadcast_to(b_attn[vs:vs + QCOLS], (128, QCOLS))).astype(np.float32)
        bp = np.ascontiguousarray(
            np.broadcast_to(b_proj[es:es + ESL], (128, ESL))).astype(np.float32)
        in_maps.append({
            "xT": xT[b], "wqk": wqk, "wv": wv, "wp": wp,
            "bqk": bqk, "bv": bv, "bp": bp, "tri": tri, "ones64": ones64,
        })
    return in_maps


def kernel(x, w_attn, b_attn, w_proj, b_proj):
    nc = _get_nc()
    in_maps = _make_in_maps(x, w_attn, b_attn, w_proj, b_proj)
    res = run_bass_kernel_spmd(nc, in_maps, core_ids=list(range(N_CORES)),
                               trace=bool(os.environ.get("BASS_TRACE")))
    if res.exec_time_ns is not None:
        print(f"HW exec time: {res.exec_time_ns} ns")
    out = np.empty((B, S, D), np.float32)
    for c in range(N_CORES):
        b, es = c // TP, (c % TP) * ESL
        out[b, :, es:es + ESL] = res.results[c]["out"]
    return out
